# revision 117
# baseline (speedup 1.0000x reference)
"""Trainium2 Bass kernel for nn_Aux2_46969762349381 (scatter_memory).

Computes, for embs [32, 2048, 1024] f32:
  status_probs = softmax(embs @ W_status.T + b_status)   # [B,T,5]
  flight_probs = softmax(embs @ W_flight.T + b_flight)   # [B,T,30]
  out = concat([s0, s2, s1, s4*flight, s3*flight], -1)   # [B,T,63]

Strategy (pure data parallel over batch, 8 cores; full inputs in, full
output out): each core owns 4 batches = 8192 tokens; token t = p*64 + i
(p = SBUF partition, i = token-tile index) so embs loads and out stores
are contiguous multi-KB runs per partition.

Per-core pipeline (bf16 compute; rel err ~4.3e-3 vs f32 reference):
  - loads: ALL embs tiles stream through SWDGE f32->bf16 cast DMAs on
    the gpsimd queue. Casting in flight halves the SBUF-landing bytes;
    the DMA fabric charge follows the (halved) output side, and ACT/DVE
    are freed from cast duty entirely. Measured: multiple DMA queues do
    NOT add aggregate bandwidth (~360-410 GB/s shared cap), so a single
    cast queue loses nothing.
  - PE transposes each 128x128 block (1 cyc/row bf16) into a full-bank
    PSUM tile per token-tile; ONE PSUM->SBUF copy per tile (3/8 on ACT,
    5/8 on DVE to balance against exp/assembly) builds embT. ALL of a
    batch's transposes+copies are emitted before any of its matmuls
    (embT bufs=4): the last transpose releases ldb for batch N+2's load
    ~2us earlier, closing pre-load gaps on the DMA queue.
  - logits per token-tile: [128 tok, 35] PSUM accumulator in its OWN
    2KB bank (4 in flight + 4 psT banks = all 8). The class bias comes
    first as a K=1 matmul ones[1,128]^T @ b[1,35]; then 8 accumulating
    matmuls with the embT block as the STATIONARY operand and the
    35-col W chunk moving: 35-cycle matmuls instead of 512-cycle, and
    the result is already token-major so the old [35,tok] logits pass,
    second exp layout, and exp transpose-back all disappear.
  - ACT exp straight from each PSUM accumulator to SBUF [128, 36] bf16;
    DVE does softmax normalization + outer-product scatter into
    [128, ntile, 63] BF16 (halves store bytes; kernel() upcasts to f32
    on the host), stored as ~1KB/partition runs on sync.
  - startup: ident/bias consts first (bf16 from host, no casts), batch 0
    loads split [1,1,1,1,2,2] so the first transpose starts ~2us earlier;
    drain: final 8 tiles run as 4+2+2 batches to shorten the exposed
    compute chain after the last load.

TimelineSim (which matched the grader's 97465ns baseline within 1%):
96524 -> 65415 ns. DMA is the binding resource (50.2us busy = 16MB of
cast-load output bytes + 2.9us bf16 stores at ~360GB/s-equivalent);
PE 37.7us, ACT 39.6us, DVE 39.4us. Startup ~2.3us pre-DMA and the drain
(~8us of exposed last-chain latency) are the rest, dominated by the
~0.9us/DMA semaphore-propagation constant and the exit barrier.
Rejected on measurement/model: xbar-DMA and strided transposed loads
(32B granule = descriptor-floor death), multi-queue load splitting (no
aggregate BW), fp8 logits including a 4+4 e-chunk split (fp8 copies lose
DVE's 2-byte 2x mode and double per-copy overheads; see
kernel_bf16_backup.py round notes), half-batch assembly in steady state
(DVE instruction-overhead bloat), ldb/embT deeper buffering and psT/psacc
rebalances (scheduler regressions).
"""

import os
import sys

import numpy as np

for _p in ("/opt/trn_rl_repo", "/root/.axon_site/_ro/trn_rl_repo"):
    if os.path.isdir(_p) and _p not in sys.path:
        sys.path.insert(0, _p)

from contextlib import ExitStack

import concourse.bass as bass
import concourse.tile as tile
from concourse import mybir
from concourse.bass_utils import run_bass_kernel_spmd

N_CORES = 8
B, T, E = 32, 2048, 1024
NS, NF = 5, 30
NCLS = NS + NF          # 35 combined classes
NCLSP = 36              # padded per-tile stride in bf16 PSUM (4B alignment)
OUTC = 63
P = 128                 # SBUF partitions
ECH = E // P            # 8 emb chunks of 128
GT = 4                  # token tiles (of 128 tokens) per matmul group
GTOK = GT * P           # 512 tokens per group
AG = 2                  # groups per assembly batch
F32 = mybir.dt.float32
BF16 = mybir.dt.bfloat16
EXP = mybir.ActivationFunctionType.Exp


_CTRL_INSTS = ("InstDrain", "InstNoOp", "InstEventSemaphore",
               "InstUnconditionalBranch", "InstCompareAndBranch", "InstISA")


def _split_multiwait(nc, max_waits=1):
    """Workaround for this walrus build rejecting more than one sem-wait per
    instruction (verified: even 2-wait compute ops fail codegen): move extra
    waits onto single-wait NoOps just before the instruction."""
    for bb in nc.m.functions[0].blocks:
        insts = list(bb.instructions)
        new_list = []
        changed = False
        for inst in insts:
            si = inst.sync_info
            cap = 1 if type(inst).__name__ in _CTRL_INSTS else max_waits
            if si is not None and si.on_wait and len(si.on_wait) > cap:
                waits = list(si.on_wait)
                for w in waits[:-cap]:
                    nop = mybir.InstNoOp(
                        name=nc.get_next_instruction_name(),
                        ins=[],
                        outs=[],
                        engine=inst.engine,
                        sync_info=mybir.SyncInfo(on_wait=[w], on_update=[]),
                    )
                    nc.register_instruction(nop)
                    new_list.append(nop)
                    changed = True
                inst.sync_info = mybir.SyncInfo(
                    on_wait=waits[-cap:], on_update=list(si.on_update)
                )
            new_list.append(inst)
        if changed:
            bb.instructions = new_list


def build_program(tok, passes=1):
    """Build the per-core Bass program for `tok` tokens (tok % 1024 == 0).

    passes > 1 repeats the whole steady-state body on the same data
    (idempotent) — used only for paired-difference benchmarking."""
    S = tok // P            # token tiles per core
    n_groups = S // GT
    n_batches = n_groups // AG
    NT = AG * GT            # tiles per assembly batch (8)

    nc = bass.Bass("TRN2", num_devices=N_CORES)
    embs_d = nc.dram_tensor("embs", [tok, E], F32, kind="ExternalInput")
    w_d = nc.dram_tensor("wt", [P, ECH * NCLS], BF16, kind="ExternalInput")
    b_d = nc.dram_tensor("bias", [1, NCLS], BF16, kind="ExternalInput")
    id_d = nc.dram_tensor("ident", [P, P], BF16, kind="ExternalInput")
    # bf16 output: halves store DMA bytes and lets the assembly muls run in
    # DVE 2x 16-bit mode; kernel() upcasts to f32 on the host (rounding adds
    # ~4e-3 abs vs the 1.75e-2 budget)
    out_d = nc.dram_tensor("out", [tok, OUTC], BF16, kind="ExternalOutput")

    with tile.TileContext(nc) as tc, ExitStack() as ctx:
        consts = ctx.enter_context(tc.tile_pool(name="consts", bufs=1))
        ldb_pool = ctx.enter_context(tc.tile_pool(name="ldb", bufs=2))
        embT_pool = ctx.enter_context(tc.tile_pool(name="embT", bufs=4))
        expT_pool = ctx.enter_context(tc.tile_pool(name="expT", bufs=2))
        small = ctx.enter_context(tc.tile_pool(name="small", bufs=2))
        outsb = ctx.enter_context(tc.tile_pool(name="outsb", bufs=2))
        psT_pool = ctx.enter_context(tc.tile_pool(name="psT", bufs=4, space="PSUM"))
        # per-tile [128 tok, 35] logits accumulators; each padded to a full
        # 2KB PSUM bank so concurrent accumulation groups never share a
        # zero region (4 in flight + 4 psT banks = all 8 banks)
        psacc_pool = ctx.enter_context(tc.tile_pool(name="psacc", bufs=4, space="PSUM"))

        # ident first: the transposes need it before anything else; consts
        # come in pre-converted (bf16 where used) to skip DVE casts. w_sb is
        # loaded AFTER the first embs chunk (see below) - it is only needed
        # once the first tile has been transposed, ~2.5us later.
        id_bf = consts.tile([P, P], BF16)
        nc.sync.dma_start(id_bf[:], id_d.ap())
        # bias folded into each tile's accumulation as a K=1 matmul:
        # ones[1,128]^T @ b_row[1,35] broadcasts the class bias over tokens
        ones_row = consts.tile([1, P], BF16)
        nc.vector.memset(ones_row[:], 1.0)
        b_row = consts.tile([1, NCLS], BF16)
        nc.sync.dma_start(b_row[:], b_d.ap())
        w_sb = consts.tile([P, ECH * NCLS], BF16)

        # Trigger the ACT exp table load (~2.7us) immediately so it overlaps
        # the first embs DMAs instead of stalling the first real exp.
        warm = consts.tile([NCLS, 1], F32)
        nc.scalar.activation(warm[:], id_bf[0:NCLS, 0:1], EXP)

        embs_v = embs_d.ap().rearrange("(p i) e -> p i e", p=P, i=S)
        out_v = out_d.ap().rearrange("(p i) c -> p i c", p=P, i=S)

        # batch schedule per pass: 8-tile batches mid-stream, two 4-tile
        # batches at the end so the pipeline drain after the last load is
        # half as deep (the tail was ~10us of exposed compute latency)
        # 8-tile batches mid-stream; the final 8 tiles drain as 4+2+2 so the
        # exposed compute chain after the last load is as shallow as possible
        sched = []
        t0 = 0
        while t0 < S - NT:
            sched.append((t0, NT))
            t0 += NT
        for nt in (4, 2, 2):
            nt = min(nt, S - t0)
            if nt > 0:
                sched.append((t0, nt))
                t0 += nt
        assert t0 == S

        def emit_pass2(t0, nt, gt, gtok, ag, embTs):
            # per-tile logits [128 tok, 35] accumulated with the embT block
            # stationary (LDWEIGHTS streams columns; the 35-col W is the
            # moving operand -> 35-cycle matmuls instead of 512)
            xsb = expT_pool.tile([P, nt * NCLSP], BF16, name=f"xsb{nt}")
            for g2 in range(ag):
                embT = embTs[g2]
                for c in range(gt):
                    it = g2 * gt + c
                    psa = psacc_pool.tile([P, 512], F32, name="psa")
                    nc.tensor.matmul(
                        psa[:, 0:NCLS], ones_row[:], b_row[:],
                        start=True, stop=False,
                    )
                    for j in range(ECH):
                        nc.tensor.matmul(
                            psa[:, 0:NCLS],
                            embT[:, j * gtok + c * P:j * gtok + (c + 1) * P],
                            w_sb[:, j * NCLS:(j + 1) * NCLS],
                            start=False,
                            stop=(j == ECH - 1),
                        )
                    nc.scalar.activation(
                        xsb[:, it * NCLSP:it * NCLSP + NCLS],
                        psa[:, 0:NCLS],
                        EXP,
                    )
            # ---- assembly for nt tiles (tokens p*64 + t0 .. +nt) ----
            HB = nt
            Xall = xsb[:].rearrange("p (i c) -> p i c", c=NCLSP)
            for hb in range(nt // HB):
                X = Xall[:, hb * HB:(hb + 1) * HB, :]
                sums = small.tile([P, 2 * HB], F32, name=f"sums{HB}")
                nc.vector.reduce_sum(
                    sums[:, 0:HB], X[:, :, 0:NS], axis=mybir.AxisListType.X
                )
                nc.vector.reduce_sum(
                    sums[:, HB:2 * HB], X[:, :, NS:NCLS], axis=mybir.AxisListType.X
                )
                inv = small.tile([P, 2 * HB], F32, name=f"inv{HB}")
                nc.vector.reciprocal(inv[:], sums[:])
                inv_s = inv[:, 0:HB]
                inv_f = inv[:, HB:2 * HB]
                dd = small.tile([P, HB], F32, name=f"dd{HB}")
                nc.vector.tensor_mul(dd[:], inv_s, inv_f)
                # cc in bf16 so the two big outer-product muls below have
                # all-2-byte operands (DVE 2x mode)
                cc = small.tile([P, 2 * HB], BF16, name=f"cc{HB}")
                nc.vector.tensor_mul(cc[:, 0:HB], X[:, :, 4], dd[:])  # book
                nc.vector.tensor_mul(cc[:, HB:2 * HB], X[:, :, 3], dd[:])  # change
                o_sb = outsb.tile([P, HB * OUTC], BF16, name=f"o_sb{HB}")
                O = o_sb[:].rearrange("p (i c) -> p i c", c=OUTC)
                inv_s3 = inv_s.unsqueeze(2)
                nc.vector.tensor_mul(O[:, :, 0:1], X[:, :, 0:1], inv_s3)
                nc.vector.tensor_mul(O[:, :, 1:2], X[:, :, 2:3], inv_s3)
                nc.vector.tensor_mul(O[:, :, 2:3], X[:, :, 1:2], inv_s3)
                nc.vector.tensor_mul(
                    O[:, :, 3:3 + NF],
                    X[:, :, NS:NCLS],
                    cc[:, 0:HB].unsqueeze(2).broadcast_to((P, HB, NF)),
                )
                nc.vector.tensor_mul(
                    O[:, :, 3 + NF:OUTC],
                    X[:, :, NS:NCLS],
                    cc[:, HB:2 * HB].unsqueeze(2).broadcast_to((P, HB, NF)),
                )
                nc.sync.dma_start(
                    out_v[:, t0 + hb * HB:t0 + (hb + 1) * HB, :],
                    o_sb[:].rearrange("p (i c) -> p i c", c=OUTC),
                )

        pending = []
        first = True
        for it_idx in range(len(sched) * passes):
            t0, nt = sched[it_idx % len(sched)]
            last = it_idx == len(sched) * passes - 1
            gt = min(GT, nt)
            gtok = gt * P
            ag = nt // gt
            # All loads are SWDGE f32->bf16 casts: the DMA fabric charges the
            # (halved) output bytes, so casting in-flight costs half a raw f32
            # load and frees ACT/DVE from cast duty entirely. Two 4-tile DMAs
            # per batch keep the transpose pipeline's wake-up granularity.
            ldb = ldb_pool.tile([P, nt, E], BF16, name=f"ldb{nt}")
            # Batch 0 loads in [1,1,2,4]-tile DMAs so the first transpose can
            # start ~4us earlier; steady-state batches use two 4-tile DMAs.
            splits = (1, 1, 1, 1, 2, 2) if first else (min(4, nt),) * max(nt // 4, 1)
            c0 = 0
            for w in splits:
                nc.gpsimd.dma_start(
                    ldb[:, c0:c0 + w, :],
                    embs_v[:, t0 + c0:t0 + c0 + w, :],
                )
                c0 += w
                if first:
                    # w_sb rides the sync queue behind the first embs chunk
                    nc.sync.dma_start(w_sb[:], w_d.ap())
                    first = False
            tsrc = [ldb[:, c, :].rearrange("p (j t) -> p j t", j=ECH)
                    for c in range(nt)]

            # Pass 1: ALL transposes+copies before any matmul - the last
            # transpose is what releases ldb for batch N+2's load, and doing
            # group-1's transposes before group-0's matmuls frees it ~2us
            # earlier, closing the pre-load gaps on the DMA queue.
            embTs = []
            for g2 in range(ag):
                embT = embT_pool.tile([P, ECH * gtok], BF16, name=f"embT{gt}")
                embTs.append(embT)
                embT_v = embT[:].rearrange("p (j t) -> p j t", t=gtok)
                srcs = tsrc[g2 * gt:(g2 + 1) * gt]
                for c in range(gt):
                    psT = psT_pool.tile([P, ECH * P], BF16)
                    for j in range(ECH):
                        nc.tensor.matmul(
                            psT[:, j * P:(j + 1) * P],
                            srcs[c][:, j, :],
                            id_bf[:],
                            is_transpose=True,
                        )
                    dst = embT_v[:, :, c * P:(c + 1) * P]
                    src = psT[:].rearrange("p (j t) -> p j t", j=ECH)
                    # 3/8 copies on ACT, 5/8 on DVE: ACT also carries the 8
                    # exps per batch, DVE the assembly - this evens them out
                    if (g2 * gt + c) % 8 in (0, 3, 6):
                        nc.scalar.copy(dst, src)
                    else:
                        nc.vector.tensor_copy(dst, src)
            # Software-pipeline shift: batch N's matmul/exp/assembly pass is
            # emitted AFTER batch N+1's transposes, so on PE the order is
            # [T(N+1), M(N)] - transposes never wait behind the previous
            # batch's matmuls for their copies.
            pending.append((t0, nt, gt, gtok, ag, embTs))
            if len(pending) >= 1 or last:
                emit_pass2(*pending.pop(0))
            if last:
                while pending:
                    emit_pass2(*pending.pop(0))
    _split_multiwait(nc)
    return nc


def host_inputs(W_status, b_status, W_flight, b_flight):
    bf16 = mybir.dt.np(BF16)
    W = np.concatenate([np.asarray(W_status), np.asarray(W_flight)], axis=0)
    W = np.ascontiguousarray(W, dtype=np.float32)          # [35, 1024]
    # w_host[p, j*35 + c] = W[c, j*128 + p]
    w_host = np.ascontiguousarray(
        W.T.reshape(ECH, P, NCLS).transpose(1, 0, 2).reshape(P, ECH * NCLS)
    ).astype(bf16)
    b_host = np.ascontiguousarray(
        np.concatenate([np.asarray(b_status), np.asarray(b_flight)]).reshape(1, NCLS)
    ).astype(bf16)
    ident = np.eye(P, dtype=bf16)
    return w_host, b_host, ident


_program_cache = {}


def kernel(embs, W_status, b_status, W_flight, b_flight, **run_kwargs):
    embs = np.ascontiguousarray(np.asarray(embs), dtype=np.float32)
    tok = embs.shape[0] * embs.shape[1] // N_CORES
    w_host, b_host, ident = host_inputs(W_status, b_status, W_flight, b_flight)

    nc = _program_cache.get(tok)
    if nc is None:
        nc = build_program(tok)
        _program_cache[tok] = nc

    embs_flat = embs.reshape(-1, E)
    in_maps = [
        {
            "embs": embs_flat[c * tok:(c + 1) * tok],
            "wt": w_host,
            "bias": b_host,
            "ident": ident,
        }
        for c in range(N_CORES)
    ]
    res = run_bass_kernel_spmd(
        nc, in_maps, core_ids=list(range(N_CORES)), **run_kwargs
    )
    out = np.concatenate(
        [np.asarray(res.results[c]["out"], dtype=np.float32) for c in range(N_CORES)],
        axis=0,
    )
    out = out.reshape(embs.shape[0], embs.shape[1], OUTC)
    if run_kwargs:
        return out, res
    return out

